# revision 4
# baseline (speedup 1.0000x reference)
"""Erosion (5x5 sliding-window min, geodesic border pad 1e4) on TRN2.

Layout: partition p holds rows 8p-2 .. 8p+9 of one image as 12 segments
of a bf16 tile xt16 [128, 12x1028] (2-col pads, memset once; row pads
for p0/p127 are constant and DMA'd once from a 1e4 tile).

Load path (per image): the 4MB main block (rows 8p..8p+7) rides HWDGE
f32 into xt32 [128, 8x1024] (partition-contiguous 32KB runs -> cheap
descriptors) and the Scalar engine casts it to bf16 into xt16 segs
2..9 (6.8us). The 2MB halo pair loads are gpsimd SWDGE cast-DMAs
straight into xt16 segs 0,1/10,11 (SWDGE pays ~25ns/2KB-run descriptor
generation, so keeping only the halo there costs ~13us/image on its
own queue). A full SWDGE load path measured 39us/image = the previous
bottleneck; HWDGE f32 main + ACT cast is 2x cheaper end-to-end.

Min tree on DVE, all bf16 2x except the parity-forced odd-shift op,
which is the FINAL op so its 1x cost also performs the f32 output cast:
  V: m2_s = min(x_s, x_{s+1}); m4_r = min(m2_r, m2_{r+2});
     v_r = min(m4_r, m2_{r+3})                       (all 2x)
  H: b2 = min(v, v>>2); e = min(b2, v>>4)            (2x)
     out = min(e, b2>>1)  -> f32                     (1x, odd shift)
Store: one partition-contiguous HWDGE DMA (32KB runs) on the scalar
queue. Engine budget/image: DVE 25.8us (binding), sync-q 12.6, ACT
6.8 + store-q 12.6, gpsimd-q ~13. xt32 double-buffered; of32 single
(store hides under the 23us of DVE work before the next out op).
"""

import numpy as np

import concourse.bacc as bacc
import concourse.mybir as mybir
import concourse.tile as tile
from concourse.bass_utils import run_bass_kernel_spmd

B, H, W = 32, 1024, 1024
N_CORES = 8
PER_CORE = B // N_CORES     # 4 images per core
PX = 2
PAD_VAL = 1e4
F32 = mybir.dt.float32
BF16 = mybir.dt.bfloat16
MIN = mybir.AluOpType.min
COPY = mybir.ActivationFunctionType.Copy

KR = 8                      # output rows per partition (128*8 = 1024)
SEGS = KR + 2 * PX          # 12 segments per partition
WP = W + 2 * PX             # 1028 padded width

_CACHE = {}


def build_nc(repeat: int = 1):
    nc = bacc.Bacc("TRN2", debug=False, num_devices=N_CORES)
    x = nc.dram_tensor("mask", [PER_CORE, H, W], F32, kind="ExternalInput").ap()
    y = nc.dram_tensor("out", [PER_CORE, H, W], F32, kind="ExternalOutput").ap()

    with tile.TileContext(nc) as tc:
        with (
            tc.tile_pool(name="const", bufs=1) as cpool,
            tc.tile_pool(name="x32", bufs=2) as x32p,
            tc.tile_pool(name="x16", bufs=1) as x16p,
            tc.tile_pool(name="pa", bufs=1) as pap,
            tc.tile_pool(name="pb", bufs=1) as pbp,
            tc.tile_pool(name="pv", bufs=1) as pvp,
            tc.tile_pool(name="op", bufs=1) as opool,
        ):
            # 1e4 source for row-pad fills (memset can't start at
            # partition 127; DMA is exempt from start-partition rules)
            cpad = cpool.tile([128, 2 * WP], BF16)
            nc.vector.memset(cpad[:, :], PAD_VAL)

            xt16 = x16p.tile([128, SEGS * WP], BF16)
            x16 = xt16[:, :].rearrange("p (s c) -> p s c", s=SEGS)
            # constant regions, written once: column pads + row pads
            # (p0 segs 0,1 / p127 segs 10,11 -- no per-image DMA ever
            # touches them: halo DMAs cover partitions 1..127 / 0..126
            # and the main cast covers segs 2..9 only)
            nc.vector.memset(x16[:, :, 0:PX], PAD_VAL)
            nc.vector.memset(x16[:, :, W + PX : WP], PAD_VAL)
            nc.sync.dma_start(out=x16[0:1, 0:PX, :], in_=cpad[0:1, :])
            nc.sync.dma_start(
                out=x16[127:128, KR + PX : SEGS, :], in_=cpad[0:1, :]
            )

            for rep in range(repeat):
                for img in range(PER_CORE):
                    # main rows f32 -> xt32 (contiguous 32KB runs)
                    xt32 = x32p.tile([128, KR * W], F32, tag="x")
                    nc.sync.dma_start(
                        out=xt32[:, :].rearrange("p (s c) -> p s c", s=KR),
                        in_=x[img].rearrange("(p s) c -> p s c", s=KR),
                    )
                    # halo pairs: gpsimd cast-DMA f32 -> bf16
                    nc.gpsimd.dma_start(
                        out=x16[1:128, 0:PX, PX : W + PX],
                        in_=x[img, KR - PX : H - PX, :].rearrange(
                            "(p s) c -> p s c", s=KR
                        )[:, 0:PX, :],
                    )
                    nc.gpsimd.dma_start(
                        out=x16[0:127, KR + PX : SEGS, PX : W + PX],
                        in_=x[img, KR:H, :].rearrange(
                            "(p s) c -> p s c", s=KR
                        )[:, 0:PX, :],
                    )
                    # main cast f32 -> bf16 on the Scalar engine
                    nc.scalar.activation(
                        out=x16[:, PX : PX + KR, PX : W + PX],
                        in_=xt32[:, :].rearrange("p (s c) -> p s c", s=KR),
                        func=COPY,
                    )

                    # vertical pass (segment-axis shifts, all bf16 2x)
                    w2 = pap.tile([128, (SEGS - 1) * WP], BF16, tag="a")
                    w2_3 = w2[:, :].rearrange("p (s c) -> p s c", s=SEGS - 1)
                    nc.vector.tensor_tensor(
                        out=w2_3[:, :, :],
                        in0=x16[:, 0 : SEGS - 1, :],
                        in1=x16[:, 1:SEGS, :],
                        op=MIN,
                    )
                    m4 = pbp.tile([128, KR * WP], BF16, tag="b")
                    m4_3 = m4[:, :].rearrange("p (s c) -> p s c", s=KR)
                    nc.vector.tensor_tensor(
                        out=m4_3[:, :, :],
                        in0=w2_3[:, 0:KR, :],
                        in1=w2_3[:, 2 : KR + 2, :],
                        op=MIN,
                    )
                    v = pvp.tile([128, KR * WP], BF16, tag="v")
                    v3 = v[:, :].rearrange("p (s c) -> p s c", s=KR)
                    nc.vector.tensor_tensor(
                        out=v3[:, :, :],
                        in0=m4_3[:, :, :],
                        in1=w2_3[:, 3 : KR + 3, :],
                        op=MIN,
                    )

                    # horizontal pass
                    WB = WP - 2            # 1026 cols in b2
                    b2 = pap.tile([128, KR * WB], BF16, tag="a")
                    b2_3 = b2[:, :].rearrange("p (s c) -> p s c", s=KR)
                    nc.vector.tensor_tensor(
                        out=b2_3[:, :, :],
                        in0=v3[:, :, 0:WB],
                        in1=v3[:, :, 2:WP],
                        op=MIN,
                    )
                    e = pbp.tile([128, KR * W], BF16, tag="b")
                    e3 = e[:, :].rearrange("p (s c) -> p s c", s=KR)
                    nc.vector.tensor_tensor(
                        out=e3[:, :, :],
                        in0=b2_3[:, :, 0:W],
                        in1=v3[:, :, 2 * PX : WP],
                        op=MIN,
                    )
                    # final op: odd shift (1x) + bf16 -> f32 output cast
                    of = opool.tile([128, KR * W], F32, tag="o")
                    of3 = of[:, :].rearrange("p (s c) -> p s c", s=KR)
                    nc.vector.tensor_tensor(
                        out=of3[:, :, :],
                        in0=e3[:, :, :],
                        in1=b2_3[:, :, 1 : W + 1],
                        op=MIN,
                    )

                    nc.scalar.dma_start(
                        out=y[img].rearrange("(p s) c -> p s c", s=KR),
                        in_=of3[:, :, :],
                    )

    nc.compile()
    return nc


def run(mask: np.ndarray, trace: bool = False):
    assert mask.shape == (B, 1, H, W), mask.shape
    in_dtype = mask.dtype
    mask4 = np.ascontiguousarray(
        mask.reshape(B, H, W).astype(np.float32, copy=False)
    )
    if "nc" not in _CACHE:
        _CACHE["nc"] = build_nc(1)
    nc = _CACHE["nc"]
    in_maps = [
        {"mask": mask4[i * PER_CORE : (i + 1) * PER_CORE]} for i in range(N_CORES)
    ]
    res = run_bass_kernel_spmd(nc, in_maps, list(range(N_CORES)), trace=trace)
    out = np.concatenate([res.results[i]["out"] for i in range(N_CORES)], axis=0)
    return out.reshape(B, 1, H, W).astype(in_dtype, copy=False), res


def kernel(mask: np.ndarray) -> np.ndarray:
    return run(mask)[0]
